# revision 8
# baseline (speedup 1.0000x reference)
"""Additive (Bahdanau) attention kernel for Trainium2, 8 NeuronCores.

Problem shapes (hardcoded): B=8, TQ=128, TV=256, D=512, U=256.
Sharding: data-parallel over batch B -> one batch element per core.

Per-core algorithm (all on-chip after the initial DMAs):
  w1vT[u,v]  = (values @ W1)^T           via PE (K=d chunks)
  w2qT[u,q]  = (query  @ W2)^T + (b1+b2) via PE + ACT bias
  for each block of q:
      pre[u,(c,q,v)] = w1vT[u,(c,v)] + w2qT[u,(c,q)]  (DVE broadcast add;
        a couple of q's per block are instead fused into ACT tanh bias)
      feat = tanh(pre) rounded to float32r (ACT, one big-FD instr)
      score pair matmuls (float32r, ~1 cyc/row vs 4 for fp32): V is split
        host-side into tf32-exact Vh+Vl and both accumulate into the same
        PSUM group, so score = (Vh+Vl)@feat = V@feat_r exactly; the only
        precision loss is the single tf32 rounding of tanh (~2.5e-5 rel).
        Each matmul handles a q-pair: V sits at window columns 2p,2p+1 and
        the N=512 rhs is two q's feat; even q's score lands in psum cols
        0:256, odd in 256:512; the unread half of each row is don't-care.
  attn = exp(score) (no max-sub needed; |score| <= sum|V| ~ 13), with an
  even/odd predicated select; rowsums via accum_out; context = attnT^T @
  values scaled by 1/rowsum.  Softmax+context run in two q-halves so the
  first half overlaps the second half's score phase.
  bv is dropped: softmax is shift-invariant.
"""
import sys
import numpy as np

if '/opt/trn_rl_repo' not in sys.path:
    sys.path.insert(0, '/opt/trn_rl_repo')

B, TQ, TV, D, U = 8, 128, 256, 512, 256
P = 128          # partitions
KD = D // P      # 4 k-chunks over d
CU = U // P      # 2 chunks over u
CV = TV // P     # 2 chunks over v
BLOCKS = [16] * 6 + [8] * 4          # q-block sizes (smaller at the tail)
assert sum(BLOCKS) == TQ

_compiled = None


def _build():
    import concourse.bass as bass
    import concourse.tile as tile
    from concourse import bacc, mybir

    f32 = mybir.dt.float32
    AF = mybir.ActivationFunctionType

    nc = bacc.Bacc("TRN2", target_bir_lowering=False, debug=False,
                   enable_asserts=True, num_devices=B)

    W1_d = nc.dram_tensor("W1", [D, U], f32, kind="ExternalInput").ap()
    W2_d = nc.dram_tensor("W2", [D, U], f32, kind="ExternalInput").ap()
    QT_d = nc.dram_tensor("QT", [D, TQ], f32, kind="ExternalInput").ap()
    VT_d = nc.dram_tensor("VT", [D, TV], f32, kind="ExternalInput").ap()
    VAL_d = nc.dram_tensor("VAL", [TV, D], f32, kind="ExternalInput").ap()
    VWH_d = nc.dram_tensor("VWH", [P, CU, 256], f32, kind="ExternalInput").ap()
    VWL_d = nc.dram_tensor("VWL", [P, CU, 256], f32, kind="ExternalInput").ap()
    B12_d = nc.dram_tensor("B12", [P, CU], f32, kind="ExternalInput").ap()
    ID_d = nc.dram_tensor("ID", [P, P], f32, kind="ExternalInput").ap()
    ME_d = nc.dram_tensor("ME", [P, 1], mybir.dt.uint8,
                          kind="ExternalInput").ap()
    OUT_d = nc.dram_tensor("OUT", [TQ, D], f32, kind="ExternalOutput").ap()

    with tile.TileContext(nc) as tc:
        with (
            tc.tile_pool(name="cst", bufs=1) as cst,
            tc.tile_pool(name="pre_p", bufs=2) as pre_p,
            tc.tile_pool(name="feat_p", bufs=2) as feat_p,
            tc.tile_pool(name="sm", bufs=1) as sm,
            tc.tile_pool(name="ps", bufs=1, space=bass.MemorySpace.PSUM) as ps,
        ):
            f32r = mybir.dt.float32r
            # ---- inputs; chunk the projection operands so matmuls can
            # start before the full tensors arrive ----
            b12 = cst.tile([P, CU], f32, tag="b12")
            nc.sync.dma_start(b12[:], B12_d)
            w1 = cst.tile([P, KD, U], f32, tag="w1")
            vt = cst.tile([P, KD, TV], f32, tag="vt")
            w2 = cst.tile([P, KD, U], f32, tag="w2")
            qt = cst.tile([P, KD, TQ], f32, tag="qt")
            W1r = W1_d.rearrange("(k p) u -> p k u", p=P)
            VTr = VT_d.rearrange("(k p) v -> p k v", p=P)
            W2r = W2_d.rearrange("(k p) u -> p k u", p=P)
            QTr = QT_d.rearrange("(k p) q -> p k q", p=P)
            for k in range(KD):
                nc.sync.dma_start(w1[:, k, :], W1r[:, k, :])
                nc.sync.dma_start(vt[:, k, :], VTr[:, k, :])
                nc.sync.dma_start(w2[:, k, :], W2r[:, k, :])
                nc.sync.dma_start(qt[:, k, :], QTr[:, k, :])
            val = cst.tile([P, CV, D], f32, tag="val")
            nc.sync.dma_start(val[:], VAL_d.rearrange("(c p) d -> p c d", p=P))
            vwh_f = cst.tile([P, CU, 256], f32, tag="vwh_f")
            nc.sync.dma_start(vwh_f[:], VWH_d)
            vwl_f = cst.tile([P, CU, 256], f32, tag="vwl_f")
            nc.sync.dma_start(vwl_f[:], VWL_d)
            idt = cst.tile([P, P], f32, tag="idt")
            nc.sync.dma_start(idt[:], ID_d)
            mev = cst.tile([P, 1], mybir.dt.uint8, tag="mev")
            nc.sync.dma_start(mev[:], ME_d)
            vwh = cst.tile([P, CU, 256], f32r, tag="vwh")
            nc.vector.tensor_copy(vwh[:], vwh_f[:])
            vwl = cst.tile([P, CU, 256], f32r, tag="vwl")
            nc.vector.tensor_copy(vwl[:], vwl_f[:])

            # ---- projections ----
            psW1 = ps.tile([P, CU, TV], f32, tag="psW1")   # one bank
            psW2 = ps.tile([P, CU, TQ], f32, tag="psW2")   # half bank
            # NB: start=True clears has_written for the WHOLE bank, so each
            # c-group's k-accumulation must complete before the next c starts.
            for c in range(CU):
                for k in range(KD):
                    nc.tensor.matmul(psW1[:, c, :],
                                     w1[:, k, c * P:(c + 1) * P],
                                     vt[:, k, :],
                                     start=(k == 0), stop=(k == KD - 1))
            for c in range(CU):
                for k in range(KD):
                    nc.tensor.matmul(psW2[:, c, :],
                                     w2[:, k, c * P:(c + 1) * P],
                                     qt[:, k, :],
                                     start=(k == 0), stop=(k == KD - 1))
            w1vT = cst.tile([P, CU, TV], f32, tag="w1vT")
            nc.scalar.copy(w1vT[:], psW1[:])
            w2qT = cst.tile([P, CU, TQ], f32, tag="w2qT")
            for c in range(CU):
                nc.scalar.activation(w2qT[:, c, :], psW2[:, c, :],
                                     AF.Identity, bias=b12[:, c:c + 1])

            # ---- score phase (two psum groups: q<64 and q>=64) ----
            score_A = ps.tile([P, 2 * TV], f32, tag="scoreA")  # one bank
            score_B = ps.tile([P, 2 * TV], f32, tag="scoreB")  # one bank
            att = sm.tile([P, TV], f32, tag="att")
            sums = sm.tile([P, 4], f32, tag="sums")
            psT = ps.tile([P, CV, P], f32, tag="psT")          # half bank
            attnT = sm.tile([P, CV, P], f32, tag="attnT")
            ctx_ps = ps.tile([P, D], f32, tag="ctx")           # one bank
            ctx = sm.tile([P, D], f32, tag="ctxsb")
            att_e = sm.tile([P, TV], f32, tag="att_e")
            att_o = sm.tile([P, TV], f32, tag="att_o")

            def softmax_context(half):
                """Softmax + transpose + context matmul for one q-half."""
                h0 = half * 64
                score_ps = score_A if half == 0 else score_B
                nc.scalar.activation(att_e[:], score_ps[:, 0:TV], AF.Exp,
                                     accum_out=sums[:, 0:1])
                nc.scalar.activation(att_o[:], score_ps[:, TV:2 * TV], AF.Exp,
                                     accum_out=sums[:, 1:2])
                nc.vector.tensor_copy(att[:], att_o[:])
                nc.vector.copy_predicated(att[:],
                                          mev[:].broadcast_to([P, TV]),
                                          att_e[:])
                nc.vector.tensor_copy(sums[:, 2:3], sums[:, 1:2])
                nc.vector.copy_predicated(sums[:, 2:3], mev[:], sums[:, 0:1])
                nc.vector.reciprocal(sums[:, 3:4], sums[:, 2:3])
                for c in range(CV):
                    nc.tensor.transpose(psT[:, c, h0:h0 + 64],
                                        att[h0:h0 + 64, c * P:(c + 1) * P],
                                        idt[h0:h0 + 64, h0:h0 + 64])
                nc.scalar.copy(attnT[:, :, h0:h0 + 64], psT[:, :, h0:h0 + 64])
                for c in range(CV):
                    nc.tensor.matmul(ctx_ps[h0:h0 + 64, :],
                                     attnT[:, c, h0:h0 + 64],
                                     val[:, c, :],
                                     start=(c == 0), stop=(c == CV - 1))
                nc.scalar.mul(ctx[h0:h0 + 64, :], ctx_ps[h0:h0 + 64, :],
                              sums[h0:h0 + 64, 3:4])
                nc.sync.dma_start(OUT_d[h0:h0 + 64, :], ctx[h0:h0 + 64, :])

            mmA = [0, (TQ // 4) * CU * 2]   # counter, total for half A
            mmB = [0, (TQ // 4) * CU * 2]
            q0 = 0
            for bq in BLOCKS:
                n_act = 2 if bq >= 16 else 1
                n_dve = bq - n_act
                pre = pre_p.tile([P, CU, 16, TV], f32, tag="pre")
                feat = feat_p.tile([P, CU, 16, TV], f32r, tag="feat")
                in0 = w1vT[:].unsqueeze(2).broadcast_to([P, CU, n_dve, TV])
                in1 = (w2qT[:, :, q0:q0 + n_dve]
                       .unsqueeze(3).broadcast_to([P, CU, n_dve, TV]))
                nc.vector.tensor_add(pre[:, :, 0:n_dve, :], in0, in1)
                nc.scalar.activation(feat[:, :, 0:n_dve, :],
                                     pre[:, :, 0:n_dve, :], AF.Tanh)
                for j in range(n_act):
                    ql = n_dve + j
                    q = q0 + ql
                    for c in range(CU):
                        nc.scalar.activation(feat[:, c, ql, :],
                                             psW1[:, c, :], AF.Tanh,
                                             bias=w2qT[:, c, q:q + 1])
                for pl in range(bq // 2):
                    q = q0 + 2 * pl
                    score_ps, mmc = (score_A, mmA) if q < 64 else (score_B, mmB)
                    for c in range(CU):
                        rhs = feat[:, c, 2 * pl:2 * pl + 2, :]
                        for w in (vwh, vwl):
                            nc.tensor.matmul(score_ps[:],
                                             w[:, c, 127 - q:255 - q],
                                             rhs,
                                             start=(mmc[0] == 0),
                                             stop=(mmc[0] == mmc[1] - 1))
                            mmc[0] += 1
                q0 += bq
                if q0 == 64:
                    softmax_context(0)
            softmax_context(1)

    nc.compile()
    return nc


def _tf32_rne(x):
    b = np.asarray(x, np.float32).view(np.uint32)
    b = (b + 0x7FF + ((b >> 12) & 1)) & np.uint32(0xFFFFF000)
    return b.view(np.float32)


def _prep_shared(W1, b1, W2, b2, V, bv):
    Vf = np.asarray(V, np.float32)[:, 0]
    Vh = _tf32_rne(Vf)
    Vl = _tf32_rne(Vf - Vh)
    Vwh = np.zeros((P, CU, 256), np.float32)
    Vwl = np.zeros((P, CU, 256), np.float32)
    for c in range(CU):
        Vwh[:, c, 127] = Vh[c * P:(c + 1) * P]
        Vwh[:, c, 128] = Vh[c * P:(c + 1) * P]
        Vwl[:, c, 127] = Vl[c * P:(c + 1) * P]
        Vwl[:, c, 128] = Vl[c * P:(c + 1) * P]
    b12 = (b1 + b2).astype(np.float32).reshape(CU, P).T.copy()
    ident = np.eye(P, dtype=np.float32)
    maskE = (1 - (np.arange(P) % 2)).astype(np.uint8).reshape(P, 1)
    return {
        "W1": np.ascontiguousarray(W1, np.float32),
        "W2": np.ascontiguousarray(W2, np.float32),
        "VWH": Vwh,
        "VWL": Vwl,
        "B12": np.ascontiguousarray(b12),
        "ID": ident,
        "ME": maskE,
    }


def kernel(query, values, W1, b1, W2, b2, V, bv, _trace=False, _tmpdir=None):
    global _compiled
    from concourse.bass_utils import run_bass_kernel_spmd

    query = np.asarray(query, np.float32)
    values = np.asarray(values, np.float32)
    shared = _prep_shared(np.asarray(W1), np.asarray(b1), np.asarray(W2),
                          np.asarray(b2), np.asarray(V), np.asarray(bv))

    if _compiled is None:
        _compiled = _build()
    nc = _compiled

    in_maps = []
    for i in range(B):
        m = dict(shared)
        m["QT"] = np.ascontiguousarray(query[i].T)
        m["VT"] = np.ascontiguousarray(values[i].T)
        m["VAL"] = np.ascontiguousarray(values[i])
        in_maps.append(m)

    kw = {}
    if _trace:
        kw.update(trace=True, tmpdir=_tmpdir)
    res = run_bass_kernel_spmd(nc, in_maps, core_ids=list(range(B)), **kw)
    out = np.stack([res.results[i]["OUT"] for i in range(B)], axis=0)
    if _trace:
        kernel._last_trace = res
    return out


# revision 9
# speedup vs baseline: 1.0134x; 1.0134x over previous
"""Additive (Bahdanau) attention kernel for Trainium2, 8 NeuronCores.

Problem shapes (hardcoded): B=8, TQ=128, TV=256, D=512, U=256.
Sharding: data-parallel over batch B -> one batch element per core.

Per-core algorithm (all on-chip after the initial DMAs):
  w1vT[u,v]  = (values @ W1)^T           via PE (K=d chunks)
  w2qT[u,q]  = (query  @ W2)^T + (b1+b2) via PE + ACT bias
  for each block of q:
      pre[u,(c,q,v)] = w1vT[u,(c,v)] + w2qT[u,(c,q)]  (DVE broadcast add;
        a couple of q's per block are instead fused into ACT tanh bias)
      feat = tanh(pre) rounded to float32r (ACT, one big-FD instr)
      score pair matmuls (float32r, ~1 cyc/row vs 4 for fp32): V is split
        host-side into tf32-exact Vh+Vl and both accumulate into the same
        PSUM group, so score = (Vh+Vl)@feat = V@feat_r exactly; the only
        precision loss is the single tf32 rounding of tanh (~2.5e-5 rel).
        Each matmul handles a q-pair: V sits at window columns 2p,2p+1 and
        the N=512 rhs is two q's feat; even q's score lands in psum cols
        0:256, odd in 256:512; the unread half of each row is don't-care.
  attn = exp(score) (no max-sub needed; |score| <= sum|V| ~ 13), with an
  even/odd predicated select; rowsums via accum_out; context = attnT^T @
  values scaled by 1/rowsum.  Softmax+context run in two q-halves so the
  first half overlaps the second half's score phase.
  bv is dropped: softmax is shift-invariant.
"""
import sys
import numpy as np

if '/opt/trn_rl_repo' not in sys.path:
    sys.path.insert(0, '/opt/trn_rl_repo')

B, TQ, TV, D, U = 8, 128, 256, 512, 256
P = 128          # partitions
KD = D // P      # 4 k-chunks over d
CU = U // P      # 2 chunks over u
CV = TV // P     # 2 chunks over v
BLOCKS = [16] * 8                    # q-block sizes
assert sum(BLOCKS) == TQ

_compiled = None


def _build():
    import concourse.bass as bass
    import concourse.tile as tile
    from concourse import bacc, mybir

    f32 = mybir.dt.float32
    AF = mybir.ActivationFunctionType

    nc = bacc.Bacc("TRN2", target_bir_lowering=False, debug=False,
                   enable_asserts=True, num_devices=B)

    W1_d = nc.dram_tensor("W1", [P, KD, U], f32, kind="ExternalInput").ap()
    W2_d = nc.dram_tensor("W2", [P, KD, U], f32, kind="ExternalInput").ap()
    QT_d = nc.dram_tensor("QT", [P, KD, TQ], f32, kind="ExternalInput").ap()
    VT_d = nc.dram_tensor("VT", [P, KD, TV], f32, kind="ExternalInput").ap()
    VAL_d = nc.dram_tensor("VAL", [P, CV, D], f32, kind="ExternalInput").ap()
    VWH_d = nc.dram_tensor("VWH", [P, CU, 256], f32, kind="ExternalInput").ap()
    VWL_d = nc.dram_tensor("VWL", [P, CU, 256], f32, kind="ExternalInput").ap()
    B12_d = nc.dram_tensor("B12", [P, CU], f32, kind="ExternalInput").ap()
    ID_d = nc.dram_tensor("ID", [P, P], f32, kind="ExternalInput").ap()
    ME_d = nc.dram_tensor("ME", [P, 1], mybir.dt.uint8,
                          kind="ExternalInput").ap()
    OUT_d = nc.dram_tensor("OUT", [TQ, D], f32, kind="ExternalOutput").ap()

    with tile.TileContext(nc) as tc:
        with (
            tc.tile_pool(name="cst", bufs=1) as cst,
            tc.tile_pool(name="pre_p", bufs=2) as pre_p,
            tc.tile_pool(name="feat_p", bufs=2) as feat_p,
            tc.tile_pool(name="sm", bufs=1) as sm,
            tc.tile_pool(name="ps", bufs=1, space=bass.MemorySpace.PSUM) as ps,
        ):
            f32r = mybir.dt.float32r
            # ---- inputs; chunk the projection operands so matmuls can
            # start before the full tensors arrive ----
            b12 = cst.tile([P, CU], f32, tag="b12")
            nc.sync.dma_start(b12[:], B12_d)
            w1 = cst.tile([P, KD, U], f32, tag="w1")
            vt = cst.tile([P, KD, TV], f32, tag="vt")
            w2 = cst.tile([P, KD, U], f32, tag="w2")
            qt = cst.tile([P, KD, TQ], f32, tag="qt")
            nc.sync.dma_start(w1[:], W1_d)
            nc.sync.dma_start(vt[:], VT_d)
            nc.sync.dma_start(w2[:], W2_d)
            nc.sync.dma_start(qt[:], QT_d)
            val = cst.tile([P, CV, D], f32, tag="val")
            nc.sync.dma_start(val[:], VAL_d)
            vwh_f = cst.tile([P, CU, 256], f32, tag="vwh_f")
            nc.sync.dma_start(vwh_f[:], VWH_d)
            vwl_f = cst.tile([P, CU, 256], f32, tag="vwl_f")
            nc.sync.dma_start(vwl_f[:], VWL_d)
            idt = cst.tile([P, P], f32, tag="idt")
            nc.sync.dma_start(idt[:], ID_d)
            mev = cst.tile([P, 1], mybir.dt.uint8, tag="mev")
            nc.sync.dma_start(mev[:], ME_d)
            vwh = cst.tile([P, CU, 256], f32r, tag="vwh")
            nc.vector.tensor_copy(vwh[:], vwh_f[:])
            vwl = cst.tile([P, CU, 256], f32r, tag="vwl")
            nc.vector.tensor_copy(vwl[:], vwl_f[:])

            # ---- projections ----
            psW1 = ps.tile([P, CU, TV], f32, tag="psW1")   # one bank
            psW2 = ps.tile([P, CU, TQ], f32, tag="psW2")   # half bank
            # NB: start=True clears has_written for the WHOLE bank, so each
            # c-group's k-accumulation must complete before the next c starts.
            for c in range(CU):
                for k in range(KD):
                    nc.tensor.matmul(psW1[:, c, :],
                                     w1[:, k, c * P:(c + 1) * P],
                                     vt[:, k, :],
                                     start=(k == 0), stop=(k == KD - 1))
            for c in range(CU):
                for k in range(KD):
                    nc.tensor.matmul(psW2[:, c, :],
                                     w2[:, k, c * P:(c + 1) * P],
                                     qt[:, k, :],
                                     start=(k == 0), stop=(k == KD - 1))
            w1vT = cst.tile([P, CU, TV], f32, tag="w1vT")
            nc.scalar.copy(w1vT[:], psW1[:])
            w2qT = cst.tile([P, CU, TQ], f32, tag="w2qT")
            for c in range(CU):
                nc.scalar.activation(w2qT[:, c, :], psW2[:, c, :],
                                     AF.Identity, bias=b12[:, c:c + 1])

            # ---- score phase (two psum groups: q<64 and q>=64) ----
            score_A = ps.tile([P, 2 * TV], f32, tag="scoreA")  # one bank
            score_B = ps.tile([P, 2 * TV], f32, tag="scoreB")  # one bank
            att = sm.tile([P, TV], f32, tag="att")
            sums = sm.tile([P, 4], f32, tag="sums")
            psT = ps.tile([P, CV, P], f32, tag="psT")          # half bank
            attnT = sm.tile([P, CV, P], f32, tag="attnT")
            ctx_ps = ps.tile([P, D], f32, tag="ctx")           # one bank
            ctx = sm.tile([P, D], f32, tag="ctxsb")
            att_e = sm.tile([P, TV], f32, tag="att_e")
            att_o = sm.tile([P, TV], f32, tag="att_o")

            def softmax_context(half):
                """Softmax + transpose + context matmul for one q-half."""
                h0 = half * 64
                score_ps = score_A if half == 0 else score_B
                nc.scalar.activation(att_e[:], score_ps[:, 0:TV], AF.Exp,
                                     accum_out=sums[:, 0:1])
                nc.scalar.activation(att_o[:], score_ps[:, TV:2 * TV], AF.Exp,
                                     accum_out=sums[:, 1:2])
                nc.vector.tensor_copy(att[:], att_o[:])
                nc.vector.copy_predicated(att[:],
                                          mev[:].broadcast_to([P, TV]),
                                          att_e[:])
                nc.vector.tensor_copy(sums[:, 2:3], sums[:, 1:2])
                nc.vector.copy_predicated(sums[:, 2:3], mev[:], sums[:, 0:1])
                nc.vector.reciprocal(sums[:, 3:4], sums[:, 2:3])
                for c in range(CV):
                    nc.tensor.transpose(psT[:, c, h0:h0 + 64],
                                        att[h0:h0 + 64, c * P:(c + 1) * P],
                                        idt[h0:h0 + 64, h0:h0 + 64])
                nc.scalar.copy(attnT[:, :, h0:h0 + 64], psT[:, :, h0:h0 + 64])
                for c in range(CV):
                    nc.tensor.matmul(ctx_ps[h0:h0 + 64, :],
                                     attnT[:, c, h0:h0 + 64],
                                     val[:, c, :],
                                     start=(c == 0), stop=(c == CV - 1))
                nc.scalar.mul(ctx[h0:h0 + 64, :], ctx_ps[h0:h0 + 64, :],
                              sums[h0:h0 + 64, 3:4])
                nc.sync.dma_start(OUT_d[h0:h0 + 64, :], ctx[h0:h0 + 64, :])

            mmA = [0, (TQ // 4) * CU * 2]   # counter, total for half A
            mmB = [0, (TQ // 4) * CU * 2]
            q0 = 0
            for bq in BLOCKS:
                n_act = 2 if bq >= 16 else 1
                n_dve = bq - n_act
                pre = pre_p.tile([P, CU, 16, TV], f32, tag="pre")
                feat = feat_p.tile([P, CU, 16, TV], f32r, tag="feat")
                in0 = w1vT[:].unsqueeze(2).broadcast_to([P, CU, n_dve, TV])
                in1 = (w2qT[:, :, q0:q0 + n_dve]
                       .unsqueeze(3).broadcast_to([P, CU, n_dve, TV]))
                nc.vector.tensor_add(pre[:, :, 0:n_dve, :], in0, in1)
                nc.scalar.activation(feat[:, :, 0:n_dve, :],
                                     pre[:, :, 0:n_dve, :], AF.Tanh)
                for j in range(n_act):
                    ql = n_dve + j
                    q = q0 + ql
                    for c in range(CU):
                        nc.scalar.activation(feat[:, c, ql, :],
                                             psW1[:, c, :], AF.Tanh,
                                             bias=w2qT[:, c, q:q + 1])
                for pl in range(bq // 2):
                    q = q0 + 2 * pl
                    score_ps, mmc = (score_A, mmA) if q < 64 else (score_B, mmB)
                    for c in range(CU):
                        rhs = feat[:, c, 2 * pl:2 * pl + 2, :]
                        for w in (vwh, vwl):
                            nc.tensor.matmul(score_ps[:],
                                             w[:, c, 127 - q:255 - q],
                                             rhs,
                                             start=(mmc[0] == 0),
                                             stop=(mmc[0] == mmc[1] - 1))
                            mmc[0] += 1
                q0 += bq
                if q0 == 64:
                    softmax_context(0)
            softmax_context(1)

    nc.compile()
    return nc


def _tf32_rne(x):
    b = np.asarray(x, np.float32).view(np.uint32)
    b = (b + 0x7FF + ((b >> 12) & 1)) & np.uint32(0xFFFFF000)
    return b.view(np.float32)


def _prep_shared(W1, b1, W2, b2, V, bv):
    Vf = np.asarray(V, np.float32)[:, 0]
    Vh = _tf32_rne(Vf)
    Vl = _tf32_rne(Vf - Vh)
    Vwh = np.zeros((P, CU, 256), np.float32)
    Vwl = np.zeros((P, CU, 256), np.float32)
    for c in range(CU):
        Vwh[:, c, 127] = Vh[c * P:(c + 1) * P]
        Vwh[:, c, 128] = Vh[c * P:(c + 1) * P]
        Vwl[:, c, 127] = Vl[c * P:(c + 1) * P]
        Vwl[:, c, 128] = Vl[c * P:(c + 1) * P]
    b12 = (b1 + b2).astype(np.float32).reshape(CU, P).T.copy()
    ident = np.eye(P, dtype=np.float32)
    maskE = (1 - (np.arange(P) % 2)).astype(np.uint8).reshape(P, 1)
    W1c = np.ascontiguousarray(
        np.asarray(W1, np.float32).reshape(KD, P, U).transpose(1, 0, 2))
    W2c = np.ascontiguousarray(
        np.asarray(W2, np.float32).reshape(KD, P, U).transpose(1, 0, 2))
    return {
        "W1": W1c,
        "W2": W2c,
        "VWH": Vwh,
        "VWL": Vwl,
        "B12": np.ascontiguousarray(b12),
        "ID": ident,
        "ME": maskE,
    }


def kernel(query, values, W1, b1, W2, b2, V, bv, _trace=False, _tmpdir=None):
    global _compiled
    from concourse.bass_utils import run_bass_kernel_spmd

    query = np.asarray(query, np.float32)
    values = np.asarray(values, np.float32)
    shared = _prep_shared(np.asarray(W1), np.asarray(b1), np.asarray(W2),
                          np.asarray(b2), np.asarray(V), np.asarray(bv))

    if _compiled is None:
        _compiled = _build()
    nc = _compiled

    in_maps = []
    for i in range(B):
        m = dict(shared)
        qT = query[i].T.reshape(KD, P, TQ).transpose(1, 0, 2)
        vT = values[i].T.reshape(KD, P, TV).transpose(1, 0, 2)
        vl = values[i].reshape(CV, P, D).transpose(1, 0, 2)
        m["QT"] = np.ascontiguousarray(qT)
        m["VT"] = np.ascontiguousarray(vT)
        m["VAL"] = np.ascontiguousarray(vl)
        in_maps.append(m)

    kw = {}
    if _trace:
        kw.update(trace=True, tmpdir=_tmpdir)
    res = run_bass_kernel_spmd(nc, in_maps, core_ids=list(range(B)), **kw)
    out = np.stack([res.results[i]["OUT"] for i in range(B)], axis=0)
    if _trace:
        kernel._last_trace = res
    return out


# revision 10
# speedup vs baseline: 1.0260x; 1.0125x over previous
"""Additive (Bahdanau) attention kernel for Trainium2, 8 NeuronCores.

Problem shapes (hardcoded): B=8, TQ=128, TV=256, D=512, U=256.
Sharding: data-parallel over batch B -> one batch element per core.

Per-core algorithm (all on-chip after the initial DMAs):
  w1vT[u,v]  = (values @ W1)^T           via PE (K=d chunks)
  w2qT[u,q]  = (query  @ W2)^T + (b1+b2) via PE + ACT bias
  for each block of q:
      pre[u,(c,q,v)] = w1vT[u,(c,v)] + w2qT[u,(c,q)]  (DVE broadcast add;
        a couple of q's per block are instead fused into ACT tanh bias)
      feat = tanh(pre) rounded to float32r (ACT, one big-FD instr)
      score pair matmuls (float32r, ~1 cyc/row vs 4 for fp32): V is split
        host-side into tf32-exact Vh+Vl and both accumulate into the same
        PSUM group, so score = (Vh+Vl)@feat = V@feat_r exactly; the only
        precision loss is the single tf32 rounding of tanh (~2.5e-5 rel).
        Each matmul handles a q-pair: V sits at window columns 2p,2p+1 and
        the N=512 rhs is two q's feat; even q's score lands in psum cols
        0:256, odd in 256:512; the unread half of each row is don't-care.
  attn = exp(score) (no max-sub needed; |score| <= sum|V| ~ 13), with an
  even/odd predicated select; rowsums via accum_out; context = attnT^T @
  values scaled by 1/rowsum.  Softmax+context run in two q-halves so the
  first half overlaps the second half's score phase.
  bv is dropped: softmax is shift-invariant.
"""
import sys
import numpy as np

if '/opt/trn_rl_repo' not in sys.path:
    sys.path.insert(0, '/opt/trn_rl_repo')

B, TQ, TV, D, U = 8, 128, 256, 512, 256
P = 128          # partitions
KD = D // P      # 4 k-chunks over d
CU = U // P      # 2 chunks over u
CV = TV // P     # 2 chunks over v
BLOCKS = [16] * 7 + [8] * 2          # q-block sizes (short tail)
assert sum(BLOCKS) == TQ

_compiled = None


def _build():
    import concourse.bass as bass
    import concourse.tile as tile
    from concourse import bacc, mybir

    f32 = mybir.dt.float32
    AF = mybir.ActivationFunctionType

    nc = bacc.Bacc("TRN2", target_bir_lowering=False, debug=False,
                   enable_asserts=True, num_devices=B)

    W1_d = nc.dram_tensor("W1", [P, KD, U], f32, kind="ExternalInput").ap()
    W2_d = nc.dram_tensor("W2", [P, KD, U], f32, kind="ExternalInput").ap()
    QT_d = nc.dram_tensor("QT", [P, KD, TQ], f32, kind="ExternalInput").ap()
    VT_d = nc.dram_tensor("VT", [P, KD, TV], f32, kind="ExternalInput").ap()
    VAL_d = nc.dram_tensor("VAL", [P, CV, D], f32, kind="ExternalInput").ap()
    VWH_d = nc.dram_tensor("VWH", [P, CU, 256], f32, kind="ExternalInput").ap()
    VWL_d = nc.dram_tensor("VWL", [P, CU, 256], f32, kind="ExternalInput").ap()
    B12_d = nc.dram_tensor("B12", [P, CU], f32, kind="ExternalInput").ap()
    ID_d = nc.dram_tensor("ID", [P, P], f32, kind="ExternalInput").ap()
    ME_d = nc.dram_tensor("ME", [P, 1], mybir.dt.uint8,
                          kind="ExternalInput").ap()
    OUT_d = nc.dram_tensor("OUT", [TQ, D], f32, kind="ExternalOutput").ap()

    with tile.TileContext(nc) as tc:
        with (
            tc.tile_pool(name="cst", bufs=1) as cst,
            tc.tile_pool(name="pre_p", bufs=2) as pre_p,
            tc.tile_pool(name="feat_p", bufs=2) as feat_p,
            tc.tile_pool(name="sm", bufs=1) as sm,
            tc.tile_pool(name="ps", bufs=1, space=bass.MemorySpace.PSUM) as ps,
        ):
            f32r = mybir.dt.float32r
            # ---- inputs; chunk the projection operands so matmuls can
            # start before the full tensors arrive ----
            b12 = cst.tile([P, CU], f32, tag="b12")
            nc.gpsimd.dma_start(b12[:], B12_d)
            w1 = cst.tile([P, KD, U], f32, tag="w1")
            vt = cst.tile([P, KD, TV], f32, tag="vt")
            w2 = cst.tile([P, KD, U], f32, tag="w2")
            qt = cst.tile([P, KD, TQ], f32, tag="qt")
            nc.sync.dma_start(w1[:], W1_d)
            nc.sync.dma_start(vt[:], VT_d)
            nc.sync.dma_start(w2[:], W2_d)
            nc.sync.dma_start(qt[:], QT_d)
            val = cst.tile([P, CV, D], f32, tag="val")
            nc.sync.dma_start(val[:], VAL_d)
            vwh_f = cst.tile([P, CU, 256], f32, tag="vwh_f")
            nc.gpsimd.dma_start(vwh_f[:], VWH_d)
            vwl_f = cst.tile([P, CU, 256], f32, tag="vwl_f")
            nc.gpsimd.dma_start(vwl_f[:], VWL_d)
            idt = cst.tile([P, P], f32, tag="idt")
            nc.gpsimd.dma_start(idt[:], ID_d)
            mev = cst.tile([P, 1], mybir.dt.uint8, tag="mev")
            nc.gpsimd.dma_start(mev[:], ME_d)
            vwh = cst.tile([P, CU, 256], f32r, tag="vwh")
            nc.vector.tensor_copy(vwh[:], vwh_f[:])
            vwl = cst.tile([P, CU, 256], f32r, tag="vwl")
            nc.vector.tensor_copy(vwl[:], vwl_f[:])

            # ---- projections ----
            psW1 = ps.tile([P, CU, TV], f32, tag="psW1")   # one bank
            psW2 = ps.tile([P, CU, TQ], f32, tag="psW2")   # half bank
            # NB: start=True clears has_written for the WHOLE bank, so each
            # c-group's k-accumulation must complete before the next c starts.
            for c in range(CU):
                for k in range(KD):
                    nc.tensor.matmul(psW1[:, c, :],
                                     w1[:, k, c * P:(c + 1) * P],
                                     vt[:, k, :],
                                     start=(k == 0), stop=(k == KD - 1))
            w1vT = cst.tile([P, CU, TV], f32, tag="w1vT")
            nc.scalar.copy(w1vT[:], psW1[:])
            w2qT = cst.tile([P, CU, TQ], f32, tag="w2qT")
            for qh in range(2):
                qs = slice(qh * 64, qh * 64 + 64)
                for c in range(CU):
                    for k in range(KD):
                        nc.tensor.matmul(psW2[:, c, qs],
                                         w2[:, k, c * P:(c + 1) * P],
                                         qt[:, k, qs],
                                         start=(k == 0), stop=(k == KD - 1))
                for c in range(CU):
                    nc.scalar.activation(w2qT[:, c, qs], psW2[:, c, qs],
                                         AF.Identity, bias=b12[:, c:c + 1])

            # ---- score phase (two psum groups: q<64 and q>=64) ----
            score_A = ps.tile([P, 2 * TV], f32, tag="scoreA")  # one bank
            score_B = ps.tile([P, 2 * TV], f32, tag="scoreB")  # one bank
            att = sm.tile([P, TV], f32, tag="att")
            sums = sm.tile([P, 4], f32, tag="sums")
            psT = ps.tile([P, CV, P], f32, tag="psT")          # half bank
            attnT = sm.tile([P, CV, P], f32, tag="attnT")
            ctx_ps = ps.tile([P, D], f32, tag="ctx")           # one bank
            ctx = sm.tile([P, D], f32, tag="ctxsb")
            att_e = sm.tile([P, TV], f32, tag="att_e")
            att_o = sm.tile([P, TV], f32, tag="att_o")

            def softmax_context(half):
                """Softmax + transpose + context matmul for one q-half."""
                h0 = half * 64
                score_ps = score_A if half == 0 else score_B
                nc.scalar.activation(att_e[:], score_ps[:, 0:TV], AF.Exp,
                                     accum_out=sums[:, 0:1])
                nc.scalar.activation(att_o[:], score_ps[:, TV:2 * TV], AF.Exp,
                                     accum_out=sums[:, 1:2])
                nc.vector.tensor_copy(att[:], att_o[:])
                nc.vector.copy_predicated(att[:],
                                          mev[:].broadcast_to([P, TV]),
                                          att_e[:])
                nc.vector.tensor_copy(sums[:, 2:3], sums[:, 1:2])
                nc.vector.copy_predicated(sums[:, 2:3], mev[:], sums[:, 0:1])
                nc.vector.reciprocal(sums[:, 3:4], sums[:, 2:3])
                for c in range(CV):
                    nc.tensor.transpose(psT[:, c, h0:h0 + 64],
                                        att[h0:h0 + 64, c * P:(c + 1) * P],
                                        idt[h0:h0 + 64, h0:h0 + 64])
                nc.scalar.copy(attnT[:, :, h0:h0 + 64], psT[:, :, h0:h0 + 64])
                for c in range(CV):
                    nc.tensor.matmul(ctx_ps[h0:h0 + 64, :],
                                     attnT[:, c, h0:h0 + 64],
                                     val[:, c, :],
                                     start=(c == 0), stop=(c == CV - 1))
                nc.scalar.mul(ctx[h0:h0 + 64, :], ctx_ps[h0:h0 + 64, :],
                              sums[h0:h0 + 64, 3:4])
                nc.sync.dma_start(OUT_d[h0:h0 + 64, :], ctx[h0:h0 + 64, :])

            mmA = [0, (TQ // 4) * CU * 2]   # counter, total for half A
            mmB = [0, (TQ // 4) * CU * 2]
            q0 = 0
            for bq in BLOCKS:
                n_act = 2 if bq >= 16 else 1
                n_dve = bq - n_act
                pre = pre_p.tile([P, CU, 16, TV], f32, tag="pre")
                feat = feat_p.tile([P, CU, 16, TV], f32r, tag="feat")
                in0 = w1vT[:].unsqueeze(2).broadcast_to([P, CU, n_dve, TV])
                in1 = (w2qT[:, :, q0:q0 + n_dve]
                       .unsqueeze(3).broadcast_to([P, CU, n_dve, TV]))
                nc.vector.tensor_add(pre[:, :, 0:n_dve, :], in0, in1)
                nc.scalar.activation(feat[:, :, 0:n_dve, :],
                                     pre[:, :, 0:n_dve, :], AF.Tanh)
                for j in range(n_act):
                    ql = n_dve + j
                    q = q0 + ql
                    for c in range(CU):
                        nc.scalar.activation(feat[:, c, ql, :],
                                             psW1[:, c, :], AF.Tanh,
                                             bias=w2qT[:, c, q:q + 1])
                for pl in range(bq // 2):
                    q = q0 + 2 * pl
                    score_ps, mmc = (score_A, mmA) if q < 64 else (score_B, mmB)
                    for c in range(CU):
                        rhs = feat[:, c, 2 * pl:2 * pl + 2, :]
                        for w in (vwh, vwl):
                            nc.tensor.matmul(score_ps[:],
                                             w[:, c, 127 - q:255 - q],
                                             rhs,
                                             start=(mmc[0] == 0),
                                             stop=(mmc[0] == mmc[1] - 1))
                            mmc[0] += 1
                q0 += bq
                if q0 == 64:
                    softmax_context(0)
            softmax_context(1)

    nc.compile()
    return nc


def _tf32_rne(x):
    b = np.asarray(x, np.float32).view(np.uint32)
    b = (b + 0x7FF + ((b >> 12) & 1)) & np.uint32(0xFFFFF000)
    return b.view(np.float32)


def _prep_shared(W1, b1, W2, b2, V, bv):
    Vf = np.asarray(V, np.float32)[:, 0]
    Vh = _tf32_rne(Vf)
    Vl = _tf32_rne(Vf - Vh)
    Vwh = np.zeros((P, CU, 256), np.float32)
    Vwl = np.zeros((P, CU, 256), np.float32)
    for c in range(CU):
        Vwh[:, c, 127] = Vh[c * P:(c + 1) * P]
        Vwh[:, c, 128] = Vh[c * P:(c + 1) * P]
        Vwl[:, c, 127] = Vl[c * P:(c + 1) * P]
        Vwl[:, c, 128] = Vl[c * P:(c + 1) * P]
    b12 = (b1 + b2).astype(np.float32).reshape(CU, P).T.copy()
    ident = np.eye(P, dtype=np.float32)
    maskE = (1 - (np.arange(P) % 2)).astype(np.uint8).reshape(P, 1)
    W1c = np.ascontiguousarray(
        np.asarray(W1, np.float32).reshape(KD, P, U).transpose(1, 0, 2))
    W2c = np.ascontiguousarray(
        np.asarray(W2, np.float32).reshape(KD, P, U).transpose(1, 0, 2))
    return {
        "W1": W1c,
        "W2": W2c,
        "VWH": Vwh,
        "VWL": Vwl,
        "B12": np.ascontiguousarray(b12),
        "ID": ident,
        "ME": maskE,
    }


def kernel(query, values, W1, b1, W2, b2, V, bv, _trace=False, _tmpdir=None):
    global _compiled
    from concourse.bass_utils import run_bass_kernel_spmd

    query = np.asarray(query, np.float32)
    values = np.asarray(values, np.float32)
    shared = _prep_shared(np.asarray(W1), np.asarray(b1), np.asarray(W2),
                          np.asarray(b2), np.asarray(V), np.asarray(bv))

    if _compiled is None:
        _compiled = _build()
    nc = _compiled

    in_maps = []
    for i in range(B):
        m = dict(shared)
        qT = query[i].T.reshape(KD, P, TQ).transpose(1, 0, 2)
        vT = values[i].T.reshape(KD, P, TV).transpose(1, 0, 2)
        vl = values[i].reshape(CV, P, D).transpose(1, 0, 2)
        m["QT"] = np.ascontiguousarray(qT)
        m["VT"] = np.ascontiguousarray(vT)
        m["VAL"] = np.ascontiguousarray(vl)
        in_maps.append(m)

    kw = {}
    if _trace:
        kw.update(trace=True, tmpdir=_tmpdir)
    res = run_bass_kernel_spmd(nc, in_maps, core_ids=list(range(B)), **kw)
    out = np.stack([res.results[i]["OUT"] for i in range(B)], axis=0)
    if _trace:
        kernel._last_trace = res
    return out
